# revision 18
# baseline (speedup 1.0000x reference)
"""Trainium2 Bass kernel for nn_CrossAttention (dense transformer block).

Strategy:
  - Data-parallel over batch: 16 batches / 8 cores = 2 batches per core.
    No collectives; one SPMD NEFF run on cores 0-7 with per-core inputs.
  - Activations flow feature-major on-chip (features on partitions, tokens on
    free dim); weights are host-pre-transposed/pre-tiled so every linear is a
    chain of 128-contraction matmuls accumulated in PSUM (fp32r = full PE
    rate at free-dim >= 256).
  - Attention computes both S=[q,kv] (for the A output + row sums) and
    St=[kv,q] (exp of which feeds A@V directly) — cheaper than transposing
    exp(S) on-chip; head pairs pack into the PE array concurrently via base
    partitions 0/64 (row/col grouping).
  - Softmax denominators fold in per-head: A-path scales by the reciprocal
    row sum per partition; ctx-path scales the accumulated (exp @ V) PSUM
    columns with a partition-broadcast reciprocal built via a tiny PE
    transpose + SBUF flatten DMA.
  - Mask folds into Q (masked query rows of Q zeroed -> logits 0 -> softmax
    uniform, exactly matching where(mask, logits, -1e4) under softmax).
  - exp(tau)/sqrt(dk) folds into Wq on the host.
  - SBUF pressure: QKV+attention run per batch; ctx^T spills to a DRAM
    scratch; FFN runs in 512-token blocks with the FFN hidden in bf16.
"""

import math
import os
import sys

import numpy as np

if "/opt/trn_rl_repo" not in sys.path:
    sys.path.insert(0, "/opt/trn_rl_repo")

B, NP, NH, D, H = 16, 1024, 512, 768, 12
DK = D // H          # 64
FF = 4 * D           # 3072
NCORES = 8
BC = B // NCORES     # 2 batches per core
T = BC * NP          # 2048 query tokens per core
KV = BC * NH         # 1024 kv tokens per core
KC = D // 128        # 6 contraction chunks of 128
HP = H // 2          # 6 head pairs
NBLK = 4             # FFN token blocks
BT = T // NBLK       # 512 tokens per FFN block

_CACHE = {}
LAST_RESULT = None


class _Ctx:
    pass


def _constants(g):
    import concourse.bass as bass
    nc = g.nc
    from concourse.masks import make_identity

    def bcast_ap(handle, n, p=128):
        return bass.AP(tensor=handle, offset=0, ap=[[0, p], [1, n]])

    g.ident = g.constp.tile([128, 128], g.f32, name="ident")
    make_identity(nc, g.ident)
    g.identr = g.constp.tile([128, 128], g.f32r, name="identr")
    nc.vector.tensor_copy(g.identr, g.ident)
    g.b1c = g.constp.tile([128, FF // 128], g.f32, name="b1c")
    nc.sync.dma_start(out=g.b1c, in_=g.b1_h[:].rearrange("(c p) -> p c", p=128))
    g.boutc = g.constp.tile([128, KC], g.f32, name="boutc")
    nc.sync.dma_start(out=g.boutc, in_=g.bout_h[:].rearrange("(c p) -> p c", p=128))
    g.b2b = g.constp.tile([128, D], g.f32, name="b2b")
    nc.sync.dma_start(out=g.b2b, in_=bcast_ap(g.b2_h, D))
    g.gammab = g.constp.tile([128, D], g.f32, name="gammab")
    nc.sync.dma_start(out=g.gammab, in_=bcast_ap(g.gamma_h, D))
    g.betab = g.constp.tile([128, D], g.f32, name="betab")
    nc.sync.dma_start(out=g.betab, in_=bcast_ap(g.beta_h, D))
    g.eps_t = g.constp.tile([128, 1], g.f32, name="eps_t")
    nc.vector.memset(g.eps_t, 1e-5)
    g.mbc = g.constp.tile([128, T], g.f32, name="mbc")
    nc.sync.dma_start(out=g.mbc, in_=bcast_ap(g.maskf_h, T))


def _proj_qkv_batch(g, b, pools):
    """Per-batch projections: QTm [D, NP] (masked), KT [D, NH], V [NH, D]."""
    nc, r = g.nc, g.r
    qin, hin, wqk, wvp_, qtmp, ktp, vvp = pools

    qT = []
    for kc in range(KC):
        t = qin.tile([128, NP], g.f32r, tag="qin", name="qin")
        nc.sync.dma_start(
            out=t, in_=g.qT_h[kc * 128:(kc + 1) * 128, b * NP:(b + 1) * NP])
        qT.append(t)
    hl = []
    for kc in range(KC):
        t = hin.tile([128, NH], g.f32r, tag="hin", name="hin")
        nc.sync.dma_start(
            out=t, in_=g.hlaT_h[kc * 128:(kc + 1) * 128, b * NH:(b + 1) * NH])
        hl.append(t)

    QTm = [qtmp.tile([128, NP], g.f32r, tag="QTm", name="QTm") for _ in range(KC)]
    for po in range(KC):
        for ts in range(NP // 512):
            sl = slice(ts * 512, ts * 512 + 512)
            msl = slice(b * NP + ts * 512, b * NP + ts * 512 + 512)
            ps = g.psA.tile([128, 512], g.f32, tag="psA", name="psA")
            for kc in range(KC):
                w = wqk.tile([128, 128], g.f32r, tag="wqk", name="wqk")
                nc.sync.dma_start(out=w, in_=g.wq4_h[kc, po, :, :])
                nc.tensor.matmul(ps, r(w), r(qT[kc][:, sl]),
                                 start=(kc == 0), stop=(kc == KC - 1))
            nc.vector.tensor_mul(QTm[po][:, sl], ps, g.mbc[:, msl])

    KT = [ktp.tile([128, NH], g.f32r, tag="KT", name="KT") for _ in range(KC)]
    for po in range(KC):
        ps = g.psA.tile([128, 512], g.f32, tag="psA", name="psA")
        for kc in range(KC):
            w = wqk.tile([128, 128], g.f32r, tag="wqk", name="wqk")
            nc.sync.dma_start(out=w, in_=g.wk4_h[kc, po, :, :])
            nc.tensor.matmul(ps, r(w), r(hl[kc][:, :]),
                             start=(kc == 0), stop=(kc == KC - 1))
        nc.scalar.copy(KT[po][:, :], ps)

    VV = [vvp.tile([128, D], g.bf16, tag="VV", name="VV")
          for _ in range(NH // 128)]
    for fh in range(2):
        wv = []
        for kc in range(KC):
            w = wvp_.tile([128, 384], g.f32r, tag="wv", name="wv")
            nc.sync.dma_start(out=w, in_=g.wv4_h[fh, kc, :, :])
            wv.append(w)
        for vt in range(NH // 128):
            fl = slice(fh * 384, fh * 384 + 384)
            ps = g.psA.tile([128, 512], g.f32, tag="psA", name="psA")
            for kc in range(KC):
                nc.tensor.matmul(
                    ps[:, 0:384], r(hl[kc][:, vt * 128:vt * 128 + 128]),
                    r(wv[kc]), start=(kc == 0), stop=(kc == KC - 1))
            nc.vector.tensor_copy(VV[vt][:, fl], ps[:, 0:384])
    return QTm, KT, VV


def _attn_spath(g, b, hp, QTm, KT, ep, apool, rsp, rcp):
    """S=[q,kv] matmuls -> exp -> A output; returns per-head row-sum tiles."""
    nc, r, AF = g.nc, g.r, g.AF
    rs_t = {par: rsp.tile([128, NP // 128], g.f32, tag="rs", name="rs")
            for par in range(2)}
    for qt in range(NP // 128):
        qsl = slice(qt * 128, qt * 128 + 128)
        for par in range(2):
            hh = 2 * hp + par
            rows = slice(par * 64, par * 64 + 64)
            ps = g.psB.tile([128, NH], g.f32, tag="psB", name="psB")
            nc.tensor.matmul(ps, r(QTm[hp][rows, qsl]), r(KT[hp][rows, :]),
                             start=True, stop=True)
            e_t = ep.tile([128, NH], g.f32, tag="et", name="et")
            nc.scalar.activation(
                e_t, ps, AF.Exp, accum_out=rs_t[par][:, qt:qt + 1])
            rc = rcp.tile([128, 1], g.f32, tag="rc", name="rc")
            nc.vector.reciprocal(rc, rs_t[par][:, qt:qt + 1])
            a_t = apool.tile([128, NH], g.f32, tag="at", name="at")
            nc.vector.tensor_scalar_mul(a_t, e_t, rc)
            nc.sync.dma_start(
                out=g.A_h[b, hh, qt * 128:qt * 128 + 128, :], in_=a_t)
    return rs_t


def _attn_recb(g, b, hp, rs_t, rrp, recp):
    """Row sums [128, NP//128] per head -> [128, NP] bcast reciprocal tile."""
    import concourse.bass as bass
    nc = g.nc
    for par in range(2):
        pst = g.psD.tile([128, 128], g.f32, tag="rt", name="rt")
        nc.tensor.transpose(pst[0:NP // 128, :], rs_t[par], g.ident)
        rr = rrp.tile([NP // 128, 128], g.f32, tag="rr", name="rr")
        nc.vector.tensor_copy(rr, pst[0:NP // 128, :])
        # flatten [8,128] across partitions into DRAM scratch [NP]
        nc.sync.dma_start(out=g.rscr[b, hp, par, :], in_=rr)
    rsum_b = recp.tile([128, NP], g.f32, tag="rsb", name="rsb")
    for par in range(2):
        rows = slice(par * 64, par * 64 + 64)
        bc = bass.AP(tensor=g.rscr.tensor,
                     offset=g.rscr.offset + ((b * HP + hp) * 2 + par) * NP,
                     ap=[[0, 64], [1, NP]])
        nc.sync.dma_start(out=rsum_b[rows, :], in_=bc)
    rec_b = recp.tile([128, NP], g.f32, tag="recb", name="recb")
    nc.vector.reciprocal(rec_b, rsum_b)
    if g.dbg_once:
        g.dbg_once = False
        nc.sync.dma_start(out=g.dbg_h[0], in_=rsum_b)
        nc.sync.dma_start(out=g.dbg_h[1], in_=rec_b)
        nc.sync.dma_start(out=g.dbg_h[4, 0:128, 0:8], in_=rs_t[0])
    return rec_b


def _attn_ctx(g, b, hp, QTm, KT, VV, rec_b, etp, csp):
    """St=[kv,q] matmuls -> exp -> ctx accumulation -> scaled spill to DRAM."""
    nc, r, AF = g.nc, g.r, g.AF
    for qh in range(NP // 512):
        qsl = slice(qh * 512, qh * 512 + 512)
        et = {0: [], 1: []}
        for c in range(NH // 128):
            kcs = slice(c * 128, c * 128 + 128)
            for par in range(2):
                rows = slice(par * 64, par * 64 + 64)
                ps = g.psB.tile([128, 512], g.f32, tag="psB", name="psB")
                nc.tensor.matmul(ps, r(KT[hp][rows, kcs]),
                                 r(QTm[hp][rows, qsl]),
                                 start=True, stop=True)
                e_t = etp.tile([128, 512], g.bf16, tag="ett", name="ett")
                nc.scalar.activation(e_t, ps, AF.Exp)
                et[par].append(e_t)
        ctx_ps = g.psC.tile([128, 512], g.f32, tag="ctx", name="ctx")
        for c in range(NH // 128):
            for par in range(2):
                hh = 2 * hp + par
                orows = slice(par * 64, par * 64 + 64)
                nc.tensor.matmul(
                    ctx_ps[orows, :], VV[c][:, hh * 64:hh * 64 + 64],
                    et[par][c],
                    start=(c == 0), stop=(c == NH // 128 - 1))
        stg = csp.tile([128, 512], g.f32r, tag="cstg", name="cstg")
        nc.vector.tensor_mul(stg, ctx_ps, rec_b[:, qsl])
        if g.dbg_ctx_once:
            g.dbg_ctx_once = False
            nc.sync.dma_start(out=g.dbg_h[5, :, 0:512], in_=stg.bitcast(g.f32))
        nc.sync.dma_start(
            out=g.ctxd[hp * 128:(hp + 1) * 128,
                       b * NP + qh * 512:b * NP + qh * 512 + 512],
            in_=stg)


def _phase12(g):
    tc = g.tc
    with (
        tc.tile_pool(name="qin", bufs=KC) as qin,
        tc.tile_pool(name="hin", bufs=KC) as hin,
        tc.tile_pool(name="wqk", bufs=12) as wqk,
        tc.tile_pool(name="wvp", bufs=KC) as wvp_,
        tc.tile_pool(name="QTm", bufs=KC) as qtmp,
        tc.tile_pool(name="KT", bufs=KC) as ktp,
        tc.tile_pool(name="VV", bufs=NH // 128) as vvp,
        tc.tile_pool(name="att_e", bufs=4) as ep,
        tc.tile_pool(name="att_a", bufs=4) as apool,
        tc.tile_pool(name="att_et", bufs=8) as etp,
        tc.tile_pool(name="att_rs", bufs=5) as rsp,
        tc.tile_pool(name="att_rr", bufs=3) as rrp,
        tc.tile_pool(name="att_rec", bufs=2) as recp,
        tc.tile_pool(name="att_rc", bufs=6) as rcp,
        tc.tile_pool(name="cstg", bufs=4) as csp,
        tc.tile_pool(name="psA", bufs=2, space="PSUM") as psA,
        tc.tile_pool(name="psB", bufs=3, space="PSUM") as psB,
        tc.tile_pool(name="psC", bufs=2, space="PSUM") as psC,
        tc.tile_pool(name="psD", bufs=1, space="PSUM") as psD,
    ):
        g.psA, g.psB, g.psC, g.psD = psA, psB, psC, psD
        for b in range(BC):
            QTm, KT, VV = _proj_qkv_batch(
                g, b, (qin, hin, wqk, wvp_, qtmp, ktp, vvp))
            for hp in range(HP):
                rs_t = _attn_spath(g, b, hp, QTm, KT, ep, apool, rsp, rcp)
                rec_b = _attn_recb(g, b, hp, rs_t, rrp, recp)
                _attn_ctx(g, b, hp, QTm, KT, VV, rec_b, etp, csp)


def _p3_outproj(g, blk, p3q, qrp):
    nc, r, OP = g.nc, g.r, g.OP
    bsl = slice(blk * BT, blk * BT + BT)
    q3 = []
    for kc in range(KC):
        t = p3q.tile([128, BT], g.f32, tag="q3", name="q3")
        nc.sync.dma_start(
            out=t, in_=g.qT_h[kc * 128:(kc + 1) * 128, bsl].bitcast(g.f32))
        q3.append(t)
    ctx3 = []
    for kc in range(KC):
        t = p3q.tile([128, BT], g.f32r, tag="c3", name="c3")
        nc.sync.dma_start(out=t, in_=g.ctxd[kc * 128:(kc + 1) * 128, bsl])
        ctx3.append(t)
    qrT = []
    for po in range(KC):
        qr = qrp.tile([128, BT], g.f32r, tag="qrT", name="qrT")
        ps = g.psF.tile([128, 512], g.f32, tag="psF", name="psF")
        for kc in range(KC):
            nc.tensor.matmul(
                ps, r(g.wo_t[kc][po]), r(ctx3[kc][:, :]),
                start=(kc == 0), stop=(kc == KC - 1))
        nc.vector.scalar_tensor_tensor(
            out=qr, in0=ps, scalar=g.boutc[:, po:po + 1],
            in1=q3[po][:, :], op0=OP.add, op1=OP.add)
        qrT.append(qr)
    return qrT


def _p3_ffn1(g, qrT, w1sp, hbp):
    nc, r, AF = g.nc, g.r, g.AF
    hb = []
    for grp in range(FF // 256):
        w1g = []
        for kc in range(KC):
            t = w1sp.tile([128, 256], g.f32r, tag="w1", name="w1")
            nc.sync.dma_start(out=t, in_=g.w1p_h[kc, grp, :, :])
            w1g.append(t)
        for hr2 in range(2):
            hrow = grp * 2 + hr2
            hb_t = hbp.tile([128, BT], g.bf16, tag="hb", name="hb")
            ps = g.psF.tile([128, 512], g.f32, tag="psF", name="psF")
            for kc in range(KC):
                nc.tensor.matmul(
                    ps, r(w1g[kc][:, hr2 * 128:hr2 * 128 + 128]), r(qrT[kc]),
                    start=(kc == 0), stop=(kc == KC - 1))
            nc.scalar.activation(hb_t, ps, AF.Gelu,
                                 bias=g.b1c[:, hrow:hrow + 1])
            hb.append(hb_t)
    return hb


def _p3_ffn2_ln(g, blk, qrT, hb, w2t, yp, otp, stp):
    nc, OP, AF = g.nc, g.OP, g.AF
    for tr in range(BT // 128):
        trg = blk * (BT // 128) + tr
        tsl = slice(tr * 128, tr * 128 + 128)
        pst = g.psT.tile([128, D], g.f32r, tag="ptr", name="ptr")
        for po in range(KC):
            nc.tensor.transpose(
                pst[:, po * 128:po * 128 + 128], qrT[po][:, tsl], g.identr)
        qrtok = yp.tile([128, D], g.f32, tag="qrtok", name="qrtok")
        # qr (token-major, from PSUM transpose) + b2 broadcast -> SBUF
        nc.vector.scalar_tensor_tensor(
            out=qrtok, in0=pst, scalar=0.0, in1=g.b2b,
            op0=OP.add, op1=OP.add)
        y_t = yp.tile([128, D], g.f32, tag="yt", name="yt")
        for dh in range(2):
            dsl = slice(dh * 384, dh * 384 + 384)
            ps = g.psY.tile([128, 384], g.f32, tag="psY", name="psY")
            for kc2 in range(FF // 128):
                nc.tensor.matmul(
                    ps, hb[kc2][:, tsl], w2t[kc2][:, dsl],
                    start=(kc2 == 0), stop=(kc2 == FF // 128 - 1))
            nc.vector.tensor_add(y_t[:, dsl], ps, qrtok[:, dsl])
        st = stp.tile([128, 3, 6], g.f32, tag="st", name="st")
        for sg in range(3):
            nc.vector.bn_stats(st[:, sg, :], y_t[:, sg * 256:sg * 256 + 256])
        mv = stp.tile([128, 2], g.f32, tag="mv", name="mv")
        nc.vector.bn_aggr(mv, st)
        sd = stp.tile([128, 1], g.f32, tag="sd", name="sd")
        nc.scalar.activation(sd, mv[:, 1:2], AF.Sqrt, bias=g.eps_t[:, 0:1])
        rsd = stp.tile([128, 1], g.f32, tag="rsd", name="rsd")
        nc.vector.reciprocal(rsd, sd)
        xc = yp.tile([128, D], g.f32, tag="xc", name="xc")
        nc.vector.tensor_scalar(
            out=xc, in0=y_t, scalar1=mv[:, 0:1], scalar2=rsd,
            op0=OP.subtract, op1=OP.mult)
        o_t = otp.tile([128, D], g.f32, tag="ot", name="ot")
        nc.vector.scalar_tensor_tensor(
            out=o_t, in0=xc, scalar=0.0, in1=g.gammab,
            op0=OP.add, op1=OP.mult)
        nc.vector.tensor_add(o_t, o_t, g.betab)
        nc.sync.dma_start(out=g.out_h[trg * 128:trg * 128 + 128, :], in_=o_t)


def _phase3(g):
    tc, nc = g.tc, g.nc
    with (
        tc.tile_pool(name="wo", bufs=36) as wop,
        tc.tile_pool(name="p3q", bufs=2 * KC) as p3q,
        tc.tile_pool(name="qrT", bufs=9) as qrp,
        tc.tile_pool(name="w1s", bufs=9) as w1sp,
        tc.tile_pool(name="hb", bufs=26) as hbp,
        tc.tile_pool(name="w2s", bufs=FF // 128) as w2sp,
        tc.tile_pool(name="yt", bufs=2) as yp,
        tc.tile_pool(name="ot", bufs=3) as otp,
        tc.tile_pool(name="st3", bufs=4) as stp,
        tc.tile_pool(name="psF", bufs=3, space="PSUM") as psF,
        tc.tile_pool(name="psY", bufs=2, space="PSUM") as psY,
        tc.tile_pool(name="psT", bufs=1, space="PSUM") as psT,
    ):
        g.psF, g.psY, g.psT = psF, psY, psT
        g.wo_t = []
        for kc in range(KC):
            row = []
            for po in range(KC):
                w = wop.tile([128, 128], g.f32r, tag="wo", name="wo")
                nc.sync.dma_start(out=w, in_=g.wo4_h[kc, po, :, :])
                row.append(w)
            g.wo_t.append(row)
        w2t = []
        for kc2 in range(FF // 128):
            t = w2sp.tile([128, D], g.bf16, tag="w2", name="w2")
            nc.sync.dma_start(out=t, in_=g.w2b_h[kc2 * 128:(kc2 + 1) * 128, :])
            w2t.append(t)
        for blk in range(NBLK):
            qrT = _p3_outproj(g, blk, p3q, qrp)
            hb = _p3_ffn1(g, qrT, w1sp, hbp)
            _p3_ffn2_ln(g, blk, qrT, hb, w2t, yp, otp, stp)


def _build_program():
    import concourse.bass as bass
    import concourse.mybir as mybir
    import concourse.tile as tile
    from concourse import bacc

    g = _Ctx()
    g.f32 = mybir.dt.float32
    g.f32r = mybir.dt.float32r
    g.bf16 = mybir.dt.bfloat16
    g.AF = mybir.ActivationFunctionType
    g.OP = mybir.AluOpType
    f32r = mybir.dt.float32r
    g.r = lambda ap: ap.bitcast(f32r)

    nc = bacc.Bacc(None)
    g.nc = nc
    g.dbg_once = True
    g.dbg_ctx_once = True
    f32, bf16 = g.f32, g.bf16

    g.qT_h = nc.dram_tensor("qT", [D, T], g.f32r, kind="ExternalInput")
    g.hlaT_h = nc.dram_tensor("hlaT", [D, KV], g.f32r, kind="ExternalInput")
    g.maskf_h = nc.dram_tensor("maskf", [T], f32, kind="ExternalInput")
    g.wq4_h = nc.dram_tensor("wq4", [KC, KC, 128, 128], g.f32r, kind="ExternalInput")
    g.wk4_h = nc.dram_tensor("wk4", [KC, KC, 128, 128], g.f32r, kind="ExternalInput")
    g.wv4_h = nc.dram_tensor("wv4", [2, KC, 128, 384], g.f32r, kind="ExternalInput")
    g.wo4_h = nc.dram_tensor("wo4", [KC, KC, 128, 128], g.f32r, kind="ExternalInput")
    g.w1p_h = nc.dram_tensor("w1p", [KC, FF // 256, 128, 256], g.f32r,
                             kind="ExternalInput")
    g.w2b_h = nc.dram_tensor("w2b", [FF, D], bf16, kind="ExternalInput")
    g.bout_h = nc.dram_tensor("bout", [D], f32, kind="ExternalInput")
    g.b1_h = nc.dram_tensor("b1", [FF], f32, kind="ExternalInput")
    g.b2_h = nc.dram_tensor("b2", [D], f32, kind="ExternalInput")
    g.gamma_h = nc.dram_tensor("gamma", [D], f32, kind="ExternalInput")
    g.beta_h = nc.dram_tensor("beta", [D], f32, kind="ExternalInput")

    g.A_h = nc.dram_tensor("A", [BC, H, NP, NH], f32, kind="ExternalOutput")
    g.dbg_h = nc.dram_tensor("dbg", [8, 128, NP], f32, kind="ExternalOutput")
    g.out_h = nc.dram_tensor("out", [T, D], f32, kind="ExternalOutput")

    with tile.TileContext(nc) as tc:
        g.tc = tc
        with (
            tc.tile_pool(name="const", bufs=1) as constp,
            tc.tile_pool(name="dram", bufs=1, space="DRAM") as dramp,
        ):
            g.constp = constp
            g.ctxd = dramp.tile([D, T], g.f32r, tag="ctxd", name="ctxd")
            g.rscr = dramp.tile([BC, HP, 2, NP], g.f32, tag="rscr", name="rscr")
            _constants(g)
            _phase12(g)
            _phase3(g)
    nc.compile()
    return nc


def _prep_shared(inputs):
    import ml_dtypes
    f32 = np.float32
    scale = float(np.exp(np.asarray(inputs["tau"], f32)[0])) / math.sqrt(DK)

    def tile4(wT):  # [D, D] -> [KC, KC, 128, 128] (kc, po)
        return np.ascontiguousarray(
            wT.reshape(KC, 128, KC, 128).transpose(0, 2, 1, 3))

    wqT = np.asarray(inputs["Wq"], f32).T * f32(scale)
    wkT = np.asarray(inputs["Wk"], f32).T
    wvT = np.asarray(inputs["Wv"], f32).T
    woT = np.asarray(inputs["Wout"], f32).T
    wv4 = np.ascontiguousarray(
        wvT.reshape(KC, 128, 2, 384).transpose(2, 0, 1, 3))
    w1T = np.ascontiguousarray(np.asarray(inputs["W1"], f32).T)  # [D, FF]
    w1p = np.ascontiguousarray(
        w1T.reshape(KC, 128, FF // 256, 256).transpose(0, 2, 1, 3))
    w2b = np.ascontiguousarray(
        np.asarray(inputs["W2"], f32).T).astype(ml_dtypes.bfloat16)  # [FF, D]
    return {
        "wq4": tile4(wqT), "wk4": tile4(wkT), "wv4": wv4, "wo4": tile4(woT),
        "w1p": w1p, "w2b": w2b,
        "bout": np.ascontiguousarray(np.asarray(inputs["bout"], f32)),
        "b1": np.ascontiguousarray(np.asarray(inputs["b1"], f32)),
        "b2": np.ascontiguousarray(np.asarray(inputs["b2"], f32)),
        "gamma": np.ascontiguousarray(np.asarray(inputs["gamma"], f32)),
        "beta": np.ascontiguousarray(np.asarray(inputs["beta"], f32)),
    }


def kernel(**inputs):
    global LAST_RESULT
    from concourse import bass_utils

    if "nc" not in _CACHE:
        _CACHE["nc"] = _build_program()
    nc = _CACHE["nc"]

    shared = _prep_shared(inputs)
    f32 = np.float32
    q = np.asarray(inputs["q"], f32)
    hla = np.asarray(inputs["hla"], f32)
    maskf = np.asarray(inputs["mask"]).astype(f32)

    in_maps = []
    for i in range(NCORES):
        bs = slice(BC * i, BC * (i + 1))
        qT = np.ascontiguousarray(q[bs].reshape(T, D).T)
        hlaT = np.ascontiguousarray(hla[bs].reshape(KV, D).T)
        mf = np.ascontiguousarray(maskf[bs].reshape(T))
        in_maps.append({"qT": qT, "hlaT": hlaT, "maskf": mf, **shared})

    res = bass_utils.run_bass_kernel_spmd(
        nc, in_maps, core_ids=list(range(NCORES)),
        trace=bool(int(os.environ.get("KERNEL_TRACE", "0"))),
    )
    LAST_RESULT = res

    A_full = np.concatenate([r["A"] for r in res.results], axis=0)
    A_full = A_full.reshape(B, H, NP, NH)
    out_full = np.concatenate(
        [r["out"].reshape(BC, NP, D) for r in res.results], axis=0)
    return out_full, A_full


# revision 22
# speedup vs baseline: 1.2858x; 1.2858x over previous
"""Trainium2 Bass kernel for nn_CrossAttention (dense transformer block).

Strategy:
  - Data-parallel over batch: 16 batches / 8 cores = 2 batches per core.
    No collectives; one SPMD NEFF run on cores 0-7 with per-core inputs.
  - Activations flow feature-major on-chip (features on partitions, tokens on
    free dim); weights are host-pre-transposed so every linear is a chain of
    128-contraction matmuls accumulated in PSUM (fp32r = full PE rate at
    free-dim >= 256; operands declared float32r end to end to satisfy the
    walrus rounding rule).
  - Attention computes both S=[q,kv] (for the A output + row sums) and
    St=[kv,q] (exp of which feeds A@V directly) — cheaper than transposing
    exp(S) on-chip; head pairs pack into the PE array concurrently via base
    partitions 0/64. The A@V accumulation runs in bf16 (fp32r matmuls cannot
    target PSUM base partition 64 for the odd head).
  - Softmax denominators fold in per-head: A-path scales by the reciprocal
    row sum per partition; ctx-path scales the accumulated (exp @ V) PSUM
    columns with a reciprocal broadcast built via PE transpose -> SBUF
    flatten -> DRAM bounce -> stride-0 partition-broadcast DMA.
  - Mask folds into Q (masked query rows of Q zeroed -> logits 0 -> softmax
    uniform, exactly matching where(mask, logits, -1e4) under softmax).
  - exp(tau)/sqrt(dk) folds into Wq on the host.
  - SBUF pressure: QKV+attention run per batch; ctx^T spills to a DRAM
    scratch; FFN runs in 512-token blocks with the FFN hidden in bf16.
  - A is stored bf16 on device (50 -> 25 MB/core of write traffic) and
    upcast on the host.
"""

import math
import os
import sys

import numpy as np

if "/opt/trn_rl_repo" not in sys.path:
    sys.path.insert(0, "/opt/trn_rl_repo")

B, NP, NH, D, H = 16, 1024, 512, 768, 12
DK = D // H          # 64
FF = 4 * D           # 3072
NCORES = 8
BC = B // NCORES     # 2 batches per core
T = BC * NP          # 2048 query tokens per core
KV = BC * NH         # 1024 kv tokens per core
KC = D // 128        # 6 contraction chunks of 128
HP = H // 2          # 6 head pairs
NBLK = 4             # FFN token blocks
BT = T // NBLK       # 512 tokens per FFN block

_CACHE = {}
LAST_RESULT = None


class _Ctx:
    pass


def _constants(g):
    import concourse.bass as bass
    nc = g.nc
    from concourse.masks import make_identity

    def bcast_ap(handle, n, p=128):
        return bass.AP(tensor=handle, offset=0, ap=[[0, p], [1, n]])

    g.ident = g.constp.tile([128, 128], g.f32, name="ident")
    make_identity(nc, g.ident)
    g.identr = g.constp.tile([128, 128], g.f32r, name="identr")
    nc.vector.tensor_copy(g.identr, g.ident)
    g.identb = g.constp.tile([128, 128], g.bf16, name="identb")
    nc.vector.tensor_copy(g.identb, g.ident)
    g.b1c = g.constp.tile([128, FF // 128], g.f32, name="b1c")
    nc.sync.dma_start(out=g.b1c, in_=g.b1_h[:].rearrange("(c p) -> p c", p=128))
    g.boutc = g.constp.tile([128, KC], g.f32, name="boutc")
    nc.sync.dma_start(out=g.boutc, in_=g.bout_h[:].rearrange("(c p) -> p c", p=128))
    g.b2b = g.constp.tile([128, D], g.f32, name="b2b")
    nc.sync.dma_start(out=g.b2b, in_=bcast_ap(g.b2_h, D))
    g.gammab = g.constp.tile([128, D], g.f32, name="gammab")
    nc.sync.dma_start(out=g.gammab, in_=bcast_ap(g.gamma_h, D))
    g.betab = g.constp.tile([128, D], g.f32, name="betab")
    nc.sync.dma_start(out=g.betab, in_=bcast_ap(g.beta_h, D))
    g.eps_t = g.constp.tile([128, 1], g.f32, name="eps_t")
    nc.vector.memset(g.eps_t, 1e-5)
    g.mbc = g.constp.tile([128, T], g.f32, name="mbc")
    nc.sync.dma_start(out=g.mbc, in_=bcast_ap(g.maskf_h, T))


def _load_w_chunks(g, handle, pool, tag, n=KC):
    """Load [128, width] contiguous row-chunks of a host-transposed weight."""
    ws = []
    for kc in range(n):
        t = pool.tile([128, handle.shape[1]], g.f32r, tag=tag, name=tag)
        g.nc.sync.dma_start(out=t, in_=handle[kc * 128:(kc + 1) * 128, :])
        ws.append(t)
    return ws


def _proj_qkv_batch(g, b, pools):
    """Per-batch projections: QTm [D, NP] (masked), KT [D, NH], V [NH, D]."""
    nc = g.nc
    qin, hin, wp, qtmp, ktp, vvp = pools

    qT = []
    for kc in range(KC):
        t = qin.tile([128, NP], g.f32r, tag="qin", name="qin")
        nc.sync.dma_start(
            out=t, in_=g.qT_h[kc * 128:(kc + 1) * 128, b * NP:(b + 1) * NP])
        qT.append(t)
    hl = []
    for kc in range(KC):
        t = hin.tile([128, NH], g.f32r, tag="hin", name="hin")
        nc.sync.dma_start(
            out=t, in_=g.hlaT_h[kc * 128:(kc + 1) * 128, b * NH:(b + 1) * NH])
        hl.append(t)

    wq = _load_w_chunks(g, g.wqT_h, wp, "w")
    QTm = [qtmp.tile([128, NP], g.f32r, tag="QTm", name="QTm") for _ in range(KC)]
    for po in range(KC):
        for ts in range(NP // 512):
            sl = slice(ts * 512, ts * 512 + 512)
            msl = slice(b * NP + ts * 512, b * NP + ts * 512 + 512)
            ps = g.psA.tile([128, 512], g.f32, tag="psA", name="psA")
            for kc in range(KC):
                nc.tensor.matmul(
                    ps, wq[kc][:, po * 128:po * 128 + 128], qT[kc][:, sl],
                    start=(kc == 0), stop=(kc == KC - 1))
            nc.vector.tensor_mul(QTm[po][:, sl], ps, g.mbc[:, msl])

    wk = _load_w_chunks(g, g.wkT_h, wp, "w")
    KT = [ktp.tile([128, NH], g.f32r, tag="KT", name="KT") for _ in range(KC)]
    for po in range(KC):
        ps = g.psA.tile([128, 512], g.f32, tag="psA", name="psA")
        for kc in range(KC):
            nc.tensor.matmul(
                ps, wk[kc][:, po * 128:po * 128 + 128], hl[kc][:, :],
                start=(kc == 0), stop=(kc == KC - 1))
        nc.scalar.copy(KT[po][:, :], ps)

    wv = _load_w_chunks(g, g.wvT_h, wp, "w")
    VV = [vvp.tile([128, D], g.bf16, tag="VV", name="VV")
          for _ in range(NH // 128)]
    for vt in range(NH // 128):
        for fh in range(2):
            fl = slice(fh * 384, fh * 384 + 384)
            ps = g.psA.tile([128, 512], g.f32, tag="psA", name="psA")
            for kc in range(KC):
                nc.tensor.matmul(
                    ps[:, 0:384], hl[kc][:, vt * 128:vt * 128 + 128],
                    wv[kc][:, fl], start=(kc == 0), stop=(kc == KC - 1))
            nc.vector.tensor_copy(VV[vt][:, fl], ps[:, 0:384])
    return QTm, KT, VV


def _attn_spath(g, b, hp, QTm, KT, ep, apool, rsp, rcp, atp):
    """S=[q,kv] matmuls -> exp -> normalized A (bf16): staged out to HBM and
    PE-transposed into AT tiles [kv,q] that feed A@V directly."""
    nc, AF = g.nc, g.AF
    rs_pair = rsp.tile([128, 16], g.f32, tag="rs", name="rs")
    at_sb = {}
    for half in range(2):
        stage = {par: apool.tile([128, 2048], g.bf16, tag="ast", name="ast")
                 for par in range(2)}
        for qq in range(4):
            qt = half * 4 + qq
            qsl = slice(qt * 128, qt * 128 + 128)
            for par in range(2):
                rows = slice(par * 64, par * 64 + 64)
                ps = g.psB.tile([128, NH], g.f32, tag="psB", name="psB")
                nc.tensor.matmul(ps, QTm[hp][rows, qsl], KT[hp][rows, :],
                                 start=True, stop=True)
                e_t = ep.tile([128, NH], g.f32, tag="et", name="et")
                col = par * 8 + qt
                nc.scalar.activation(
                    e_t, ps, AF.Exp, accum_out=rs_pair[:, col:col + 1])
                rc = rcp.tile([128, 1], g.f32, tag="rc", name="rc")
                nc.vector.reciprocal(rc, rs_pair[:, col:col + 1])
                nc.vector.tensor_scalar_mul(
                    stage[par][:, qq * 512:qq * 512 + 512], e_t, rc)
        for par in range(2):
            hh = 2 * hp + par
            out_ap = g.A_h[b, hh, half * 512:half * 512 + 512, :].rearrange(
                "(t p) k -> p t k", p=128)
            nc.scalar.dma_start(out=out_ap, in_=stage[par])
            # transpose A slabs -> AT [kv, q] bf16 for the A@V matmul
            for c in range(NH // 128):
                pst = g.psD.tile([128, 512], g.bf16, tag="atp", name="atp")
                for qq in range(4):
                    nc.tensor.transpose(
                        pst[:, qq * 128:qq * 128 + 128],
                        stage[par][:, qq * 512 + c * 128:qq * 512 + c * 128 + 128],
                        g.identb)
                t = atp.tile([128, 512], g.bf16, tag="at2", name="at2")
                eng = nc.vector if c % 2 == 0 else nc.scalar
                if c % 2 == 0:
                    nc.vector.tensor_copy(t, pst)
                else:
                    nc.scalar.copy(t, pst)
                at_sb[(par, half, c)] = t
    return at_sb


def _attn_ctx(g, b, hp, VV, at_sb, csp):
    """ctx = A @ V via transposed-A tiles; spill ctx^T to DRAM scratch."""
    nc = g.nc
    for qh in range(2):
        ctx_ps = g.psC.tile([128, 512], g.f32, tag="ctx", name="ctx")
        for c in range(NH // 128):
            for par in range(2):
                hh = 2 * hp + par
                orows = slice(par * 64, par * 64 + 64)
                nc.tensor.matmul(
                    ctx_ps[orows, :], VV[c][:, hh * 64:hh * 64 + 64],
                    at_sb[(par, qh, c)],
                    start=(c == 0), stop=(c == NH // 128 - 1))
        stg = csp.tile([128, 512], g.f32r, tag="cstg", name="cstg")
        nc.vector.tensor_copy(stg, ctx_ps)
        nc.scalar.dma_start(
            out=g.ctxd[hp * 128:(hp + 1) * 128,
                       b * NP + qh * 512:b * NP + qh * 512 + 512],
            in_=stg)


def _phase12(g):
    tc = g.tc
    with (
        tc.tile_pool(name="qin", bufs=KC) as qin,
        tc.tile_pool(name="hin", bufs=KC) as hin,
        tc.tile_pool(name="wp", bufs=7) as wp,
        tc.tile_pool(name="QTm", bufs=KC) as qtmp,
        tc.tile_pool(name="KT", bufs=KC) as ktp,
        tc.tile_pool(name="VV", bufs=NH // 128) as vvp,
        tc.tile_pool(name="att_e", bufs=4) as ep,
        tc.tile_pool(name="att_a", bufs=4) as apool,
        tc.tile_pool(name="att_at", bufs=10) as atp,
        tc.tile_pool(name="att_rs", bufs=3) as rsp,
        tc.tile_pool(name="att_rc", bufs=6) as rcp,
        tc.tile_pool(name="cstg", bufs=4) as csp,
        tc.tile_pool(name="psA", bufs=2, space="PSUM") as psA,
        tc.tile_pool(name="psB", bufs=2, space="PSUM") as psB,
        tc.tile_pool(name="psC", bufs=2, space="PSUM") as psC,
        tc.tile_pool(name="psD", bufs=2, space="PSUM") as psD,
    ):
        g.psA, g.psB, g.psC, g.psD = psA, psB, psC, psD
        for b in range(BC):
            QTm, KT, VV = _proj_qkv_batch(
                g, b, (qin, hin, wp, qtmp, ktp, vvp))
            for hp in range(HP):
                at_sb = _attn_spath(g, b, hp, QTm, KT, ep, apool, rsp, rcp,
                                    atp)
                _attn_ctx(g, b, hp, VV, at_sb, csp)


def _p3_outproj(g, blk, p3q, qrp):
    nc, OP = g.nc, g.OP
    bsl = slice(blk * BT, blk * BT + BT)
    q3 = []
    for kc in range(KC):
        t = p3q.tile([128, BT], g.f32, tag="q3", name="q3")
        nc.sync.dma_start(
            out=t, in_=g.qT_h[kc * 128:(kc + 1) * 128, bsl].bitcast(g.f32))
        q3.append(t)
    ctx3 = []
    for kc in range(KC):
        t = p3q.tile([128, BT], g.f32r, tag="c3", name="c3")
        nc.sync.dma_start(out=t, in_=g.ctxd[kc * 128:(kc + 1) * 128, bsl])
        ctx3.append(t)
    qrT = []
    for po in range(KC):
        qr = qrp.tile([128, BT], g.f32r, tag="qrT", name="qrT")
        ps = g.psF.tile([128, 512], g.f32, tag="psF", name="psF")
        for kc in range(KC):
            nc.tensor.matmul(
                ps, g.wo_t[kc][:, po * 128:po * 128 + 128], ctx3[kc][:, :],
                start=(kc == 0), stop=(kc == KC - 1))
        nc.vector.scalar_tensor_tensor(
            out=qr, in0=ps, scalar=g.boutc[:, po:po + 1],
            in1=q3[po][:, :], op0=OP.add, op1=OP.add)
        qrT.append(qr)
    return qrT


def _p3_ffn1(g, qrT, w1sp, hbp):
    nc, AF = g.nc, g.AF
    hb = []
    for quarter in range(4):
        w1q = []
        for kc in range(KC):
            t = w1sp.tile([128, 768], g.f32r, tag="w1", name="w1")
            nc.sync.dma_start(
                out=t,
                in_=g.w1T_h[kc * 128:(kc + 1) * 128,
                            quarter * 768:quarter * 768 + 768])
            w1q.append(t)
        for hr6 in range(6):
            hrow = quarter * 6 + hr6
            hb_t = hbp.tile([128, BT], g.bf16, tag="hb", name="hb")
            ps = g.psF.tile([128, 512], g.f32, tag="psF", name="psF")
            for kc in range(KC):
                nc.tensor.matmul(
                    ps, w1q[kc][:, hr6 * 128:hr6 * 128 + 128], qrT[kc],
                    start=(kc == 0), stop=(kc == KC - 1))
            nc.scalar.activation(hb_t, ps, AF.Gelu,
                                 bias=g.b1c[:, hrow:hrow + 1])
            hb.append(hb_t)
    return hb


def _p3_ffn2_ln(g, blk, qrT, hb, w2t, yp, otp, stp):
    nc, OP, AF = g.nc, g.OP, g.AF
    for tr in range(BT // 128):
        trg = blk * (BT // 128) + tr
        tsl = slice(tr * 128, tr * 128 + 128)
        pst = g.psT.tile([128, D], g.f32r, tag="ptr", name="ptr")
        for po in range(KC):
            nc.tensor.transpose(
                pst[:, po * 128:po * 128 + 128], qrT[po][:, tsl], g.identr)
        qrtok = yp.tile([128, D], g.f32, tag="qrtok", name="qrtok")
        nc.vector.scalar_tensor_tensor(
            out=qrtok, in0=pst.bitcast(g.f32), scalar=0.0, in1=g.b2b,
            op0=OP.add, op1=OP.add)
        y_t = yp.tile([128, D], g.f32, tag="yt", name="yt")
        for dh in range(2):
            dsl = slice(dh * 384, dh * 384 + 384)
            ps = g.psY.tile([128, 384], g.f32, tag="psY", name="psY")
            for kc2 in range(FF // 128):
                nc.tensor.matmul(
                    ps, hb[kc2][:, tsl], w2t[kc2][:, dsl],
                    start=(kc2 == 0), stop=(kc2 == FF // 128 - 1))
            nc.vector.tensor_add(y_t[:, dsl], ps, qrtok[:, dsl])
        st = stp.tile([128, 3, 6], g.f32, tag="st", name="st")
        for sg in range(3):
            nc.vector.bn_stats(st[:, sg, :], y_t[:, sg * 256:sg * 256 + 256])
        mv = stp.tile([128, 2], g.f32, tag="mv", name="mv")
        nc.vector.bn_aggr(mv, st)
        sd = stp.tile([128, 1], g.f32, tag="sd", name="sd")
        nc.scalar.activation(sd, mv[:, 1:2], AF.Sqrt, bias=g.eps_t[:, 0:1])
        rsd = stp.tile([128, 1], g.f32, tag="rsd", name="rsd")
        nc.vector.reciprocal(rsd, sd)
        xc = yp.tile([128, D], g.f32, tag="xc", name="xc")
        nc.vector.tensor_scalar(
            out=xc, in0=y_t, scalar1=mv[:, 0:1], scalar2=rsd,
            op0=OP.subtract, op1=OP.mult)
        o_t = otp.tile([128, D], g.f32, tag="ot", name="ot")
        nc.vector.scalar_tensor_tensor(
            out=o_t, in0=xc, scalar=0.0, in1=g.gammab,
            op0=OP.add, op1=OP.mult)
        nc.vector.tensor_add(o_t, o_t, g.betab)
        nc.scalar.dma_start(out=g.out_h[trg * 128:trg * 128 + 128, :], in_=o_t)


def _phase3(g):
    tc, nc = g.tc, g.nc
    with (
        tc.tile_pool(name="wo3", bufs=KC) as wop,
        tc.tile_pool(name="p3q", bufs=7) as p3q,
        tc.tile_pool(name="qrT", bufs=9) as qrp,
        tc.tile_pool(name="w1s", bufs=9) as w1sp,
        tc.tile_pool(name="hb", bufs=26) as hbp,
        tc.tile_pool(name="w2s", bufs=FF // 128) as w2sp,
        tc.tile_pool(name="yt", bufs=2) as yp,
        tc.tile_pool(name="ot", bufs=3) as otp,
        tc.tile_pool(name="st3", bufs=4) as stp,
        tc.tile_pool(name="psF", bufs=3, space="PSUM") as psF,
        tc.tile_pool(name="psY", bufs=2, space="PSUM") as psY,
        tc.tile_pool(name="psT", bufs=1, space="PSUM") as psT,
    ):
        g.psF, g.psY, g.psT = psF, psY, psT
        g.wo_t = _load_w_chunks(g, g.woT_h, wop, "wo3")
        w2t = []
        for kc2 in range(FF // 128):
            t = w2sp.tile([128, D], g.bf16, tag="w2", name="w2")
            nc.sync.dma_start(out=t, in_=g.w2b_h[kc2 * 128:(kc2 + 1) * 128, :])
            w2t.append(t)
        for blk in range(NBLK):
            qrT = _p3_outproj(g, blk, p3q, qrp)
            hb = _p3_ffn1(g, qrT, w1sp, hbp)
            _p3_ffn2_ln(g, blk, qrT, hb, w2t, yp, otp, stp)


def _build_program():
    import concourse.mybir as mybir
    import concourse.tile as tile
    from concourse import bacc

    g = _Ctx()
    g.f32 = mybir.dt.float32
    g.f32r = mybir.dt.float32r
    g.bf16 = mybir.dt.bfloat16
    g.AF = mybir.ActivationFunctionType
    g.OP = mybir.AluOpType

    nc = bacc.Bacc(None)
    g.nc = nc
    f32, bf16 = g.f32, g.bf16

    g.qT_h = nc.dram_tensor("qT", [D, T], g.f32r, kind="ExternalInput")
    g.hlaT_h = nc.dram_tensor("hlaT", [D, KV], g.f32r, kind="ExternalInput")
    g.maskf_h = nc.dram_tensor("maskf", [T], f32, kind="ExternalInput")
    g.wqT_h = nc.dram_tensor("wqT", [D, D], g.f32r, kind="ExternalInput")
    g.wkT_h = nc.dram_tensor("wkT", [D, D], g.f32r, kind="ExternalInput")
    g.wvT_h = nc.dram_tensor("wvT", [D, D], g.f32r, kind="ExternalInput")
    g.woT_h = nc.dram_tensor("woT", [D, D], g.f32r, kind="ExternalInput")
    g.w1T_h = nc.dram_tensor("w1T", [D, FF], g.f32r, kind="ExternalInput")
    g.w2b_h = nc.dram_tensor("w2b", [FF, D], bf16, kind="ExternalInput")
    g.bout_h = nc.dram_tensor("bout", [D], f32, kind="ExternalInput")
    g.b1_h = nc.dram_tensor("b1", [FF], f32, kind="ExternalInput")
    g.b2_h = nc.dram_tensor("b2", [D], f32, kind="ExternalInput")
    g.gamma_h = nc.dram_tensor("gamma", [D], f32, kind="ExternalInput")
    g.beta_h = nc.dram_tensor("beta", [D], f32, kind="ExternalInput")

    g.A_h = nc.dram_tensor("A", [BC, H, NP, NH], bf16, kind="ExternalOutput")
    g.out_h = nc.dram_tensor("out", [T, D], f32, kind="ExternalOutput")

    with tile.TileContext(nc) as tc:
        g.tc = tc
        with (
            tc.tile_pool(name="const", bufs=1) as constp,
            tc.tile_pool(name="dram", bufs=1, space="DRAM") as dramp,
        ):
            g.constp = constp
            g.ctxd = dramp.tile([D, T], g.f32r, tag="ctxd", name="ctxd")
            _constants(g)
            _phase12(g)
            _phase3(g)
    nc.compile()
    return nc


def _prep_shared(inputs):
    import ml_dtypes
    f32 = np.float32
    scale = float(np.exp(np.asarray(inputs["tau"], f32)[0])) / math.sqrt(DK)
    wqT = np.ascontiguousarray(np.asarray(inputs["Wq"], f32).T * f32(scale))
    wkT = np.ascontiguousarray(np.asarray(inputs["Wk"], f32).T)
    wvT = np.ascontiguousarray(np.asarray(inputs["Wv"], f32).T)
    woT = np.ascontiguousarray(np.asarray(inputs["Wout"], f32).T)
    w1T = np.ascontiguousarray(np.asarray(inputs["W1"], f32).T)   # [D, FF]
    w2b = np.ascontiguousarray(
        np.asarray(inputs["W2"], f32).T).astype(ml_dtypes.bfloat16)  # [FF, D]
    return {
        "wqT": wqT, "wkT": wkT, "wvT": wvT, "woT": woT,
        "w1T": w1T, "w2b": w2b,
        "bout": np.ascontiguousarray(np.asarray(inputs["bout"], f32)),
        "b1": np.ascontiguousarray(np.asarray(inputs["b1"], f32)),
        "b2": np.ascontiguousarray(np.asarray(inputs["b2"], f32)),
        "gamma": np.ascontiguousarray(np.asarray(inputs["gamma"], f32)),
        "beta": np.ascontiguousarray(np.asarray(inputs["beta"], f32)),
    }


def kernel(**inputs):
    global LAST_RESULT
    from concourse import bass_utils

    if "nc" not in _CACHE:
        _CACHE["nc"] = _build_program()
    nc = _CACHE["nc"]

    shared = _prep_shared(inputs)
    f32 = np.float32
    q = np.asarray(inputs["q"], f32)
    hla = np.asarray(inputs["hla"], f32)
    maskf = np.asarray(inputs["mask"]).astype(f32)

    in_maps = []
    for i in range(NCORES):
        bs = slice(BC * i, BC * (i + 1))
        qT = np.ascontiguousarray(q[bs].reshape(T, D).T)
        hlaT = np.ascontiguousarray(hla[bs].reshape(KV, D).T)
        mf = np.ascontiguousarray(maskf[bs].reshape(T))
        in_maps.append({"qT": qT, "hlaT": hlaT, "maskf": mf, **shared})

    res = bass_utils.run_bass_kernel_spmd(
        nc, in_maps, core_ids=list(range(NCORES)),
        trace=bool(int(os.environ.get("KERNEL_TRACE", "0"))),
    )
    LAST_RESULT = res

    A_full = np.concatenate(
        [r["A"].astype(np.float32) for r in res.results], axis=0)
    A_full = A_full.reshape(B, H, NP, NH)
    out_full = np.concatenate(
        [r["out"].reshape(BC, NP, D) for r in res.results], axis=0)
    return out_full, A_full
